# revision 43
# baseline (speedup 1.0000x reference)
"""Causal multi-head attention on 8 Trainium2 NeuronCores (Bass/Tile).

Problem: B=4, S=2048, D=1024, H=16 heads (HD=64), fp32, causal softmax.

Sharding (tensor parallel over heads): core c owns heads {2c, 2c+1}:
  - Wq/Wk/Wv column slices [D, 128], Wo row slice [128, D]
  - each core computes Q/K/V for its heads over the full batch, runs
    attention for its 8 (batch, head) pairs, and produces a partial
    output projection [B, S, D] in bf16; the host sums the 8 partials
    in fp32 (+ bo).

Device dataflow (all matmuls bf16 in / fp32 PSUM accumulate):
  - x is pre-transposed on host to xt[D, B*S] bf16 so Q/K/V projections
    are weight-stationary: Q^T[c, s] = sum_d Wq[d, c] xt[d, s]. V is
    produced transposed the same way and relaid to [s, c] via the DMA
    xbar transpose (on the ACT HWDGE ring to isolate xbar-mode flips).
  - scores are computed transposed, S^T[k, q], with the two heads packed
    onto disjoint PE row-groups (head0 K=64 at partitions 0-63, head1 at
    64-127) so both score matmuls run concurrently; one ACT exp call
    covers both heads' [128, 2, 512] chunk.
  - exp'd scores stream as the AV matmul's moving operand with [V | 1]
    stationary; the ones-column gives softmax denominators in psum row 64.
  - softmax skips max-subtraction (scaled causal scores are ~N(0,1); exp
    cannot overflow fp32).
  - causal masking: sub-diagonal blocks are never computed; diagonal
    128x128 blocks get a post-exp multiplicative 0/1 mask (GpSimd).
  - denominators: one ACT-table reciprocal per (head, batch) on [4, 512]
    rows (~1e-5 rel err; keeps the 3.3us DVE reciprocal out of the serial
    DVE stream), broadcast across 64 partitions via a selector matmul,
    then an in-place DVE multiply on the bf16 ctx^T.
  - out = ctx @ Wo_slice with ctx^T s-blocks stationary, Wo streaming.
  - engines execute streams in program order, so the emission is
    software-pipelined: scores(qc+1) before AV(qc), the previous batch's
    out-proj s-blocks sprinkled between attention units, and the
    normalize tail hidden under the next batch's projections.
"""

import numpy as np
import ml_dtypes

import concourse.bass as bass
import concourse.mybir as mybir
import concourse.tile as tile
from concourse import bacc
from concourse import bass_utils

B, S, D, H, HD = 4, 2048, 1024, 16, 64
N_CORES = 8
HPC = H // N_CORES          # heads per core = 2
CSL = HPC * HD              # per-core channel slice = 128
NSB = S // 128              # 16 s-blocks per sequence
NCH = D // 128              # 8 contraction chunks
NQC = S // 512              # 4 q-chunks of 512
BF16 = mybir.dt.bfloat16
F32 = mybir.dt.float32
EXP = mybir.ActivationFunctionType.Exp
SCALE = 1.0 / float(np.sqrt(HD))

_CACHE: dict = {}
LAST_RESULTS = None  # BassKernelResults of the most recent run (for test.py)


def _build():
    nc = bacc.Bacc("TRN2", target_bir_lowering=False, debug=False,
                   num_devices=N_CORES)
    xt_d = nc.dram_tensor("xt", [D, B * S], BF16, kind="ExternalInput")
    wq_d = nc.dram_tensor("wq", [D, CSL], BF16, kind="ExternalInput")
    wk_d = nc.dram_tensor("wk", [D, CSL], BF16, kind="ExternalInput")
    wv_d = nc.dram_tensor("wv", [D, CSL], BF16, kind="ExternalInput")
    wo_d = nc.dram_tensor("wo", [CSL, D], BF16, kind="ExternalInput")
    tri_d = nc.dram_tensor("tri", [128, 128], BF16, kind="ExternalInput")
    ident_d = nc.dram_tensor("ident", [128, 128], BF16, kind="ExternalInput")
    sel_d = nc.dram_tensor("sel", [4, 4 * 64], BF16, kind="ExternalInput")
    out_d = nc.dram_tensor("out", [B, S, D], BF16, kind="ExternalOutput")

    with tile.TileContext(nc) as tc:
        with (
            tc.tile_pool(name="const", bufs=1) as cpool,
            tc.tile_pool(name="xt", bufs=2) as xtpool,
            tc.tile_pool(name="seq", bufs=2) as seqpool,
            tc.tile_pool(name="p", bufs=1) as ppool,
            tc.tile_pool(name="small", bufs=4) as small,
            tc.tile_pool(name="outsb", bufs=8) as outsb,
            tc.tile_pool(name="ps_s", bufs=2, space="PSUM") as ps_s,
            tc.tile_pool(name="ps_mm", bufs=2, space="PSUM") as ps_mm,
            tc.tile_pool(name="ps_cacc", bufs=2, space="PSUM") as ps_cacc,
        ):
            wq_sb = cpool.tile([128, NCH, CSL], BF16)
            wk_sb = cpool.tile([128, NCH, CSL], BF16)
            wv_sb = cpool.tile([128, NCH, CSL], BF16)
            wo_sb = cpool.tile([128, D], BF16)
            tri_sb = cpool.tile([128, 128], BF16)
            ident_sb = cpool.tile([128, 128], BF16)
            sel_sb = cpool.tile([4, 4, 64], BF16)
            nc.sync.dma_start(wq_sb[:], wq_d.ap().rearrange("(c p) m -> p c m", p=128))
            nc.sync.dma_start(wk_sb[:], wk_d.ap().rearrange("(c p) m -> p c m", p=128))
            nc.sync.dma_start(wv_sb[:], wv_d.ap().rearrange("(c p) m -> p c m", p=128))
            nc.sync.dma_start(wo_sb[:], wo_d.ap())
            nc.sync.dma_start(tri_sb[:], tri_d.ap())
            nc.sync.dma_start(ident_sb[:], ident_d.ap())
            nc.sync.dma_start(sel_sb[:], sel_d.ap().rearrange("r (i m) -> r i m", m=64))

            def load_xt(b):
                xt_sb = xtpool.tile([128, NCH, S], BF16)
                for c in range(NCH):
                    nc.sync.dma_start(
                        xt_sb[:, c, :],
                        xt_d.ap()[c * 128:(c + 1) * 128, b * S:(b + 1) * S])
                return xt_sb

            def qkv_proj(xt_sb):
                qt = seqpool.tile([128, S], BF16, tag="qt")
                kt = seqpool.tile([128, S], BF16, tag="kt")
                vt = seqpool.tile([128, S], BF16, tag="vt")
                v = seqpool.tile([128, NSB, HPC, HD + 1], BF16, tag="v")
                nc.vector.memset(v[:, :, :, HD:HD + 1], 1.0)
                for sc in range(NQC):
                    qkacc = ps_s.tile([128, 2, 512], F32, tag="s", name="qkacc")
                    vacc = ps_mm.tile([128, 512], F32, tag="mm", name="vacc")
                    for c in range(NCH):
                        for i, w_sb in enumerate((wq_sb, wk_sb)):
                            nc.tensor.matmul(
                                qkacc[:, i, :],
                                w_sb[:, c, :],
                                xt_sb[:, c, sc * 512:(sc + 1) * 512],
                                start=(c == 0), stop=(c == NCH - 1))
                        nc.tensor.matmul(
                            vacc[:],
                            wv_sb[:, c, :],
                            xt_sb[:, c, sc * 512:(sc + 1) * 512],
                            start=(c == 0), stop=(c == NCH - 1))
                    for i, dst in enumerate((qt, kt)):
                        nc.vector.tensor_copy(
                            dst[:, sc * 512:(sc + 1) * 512], qkacc[:, i, :])
                    nc.vector.tensor_copy(
                        vt[:, sc * 512:(sc + 1) * 512], vacc[:])
                # V^T -> V via PE transpose + DVE copy into [s, h, 65] layout
                for sb in range(NSB):
                    tp = ps_mm.tile([128, 128], BF16, tag="mm")
                    nc.tensor.transpose(tp[:], vt[:, sb * 128:(sb + 1) * 128],
                                        ident_sb[:])
                    nc.vector.tensor_copy(
                        v[:, sb, :, 0:HD],
                        tp[:].rearrange("p (h e) -> p h e", h=HPC))
                return qt, kt, v

            # p tiles alternate between two tags (qc0/qc2 and qc1/qc3) so
            # only ~2 q-chunks of exp'd scores are resident at once.
            PTAGS = {0: ("pA", 12), 1: ("pB", 16), 2: ("pA", 12), 3: ("pB", 16)}

            def attn_scores(qt, kt, qc):
                nki = 4 * qc + 4   # causal: k-blocks 0 .. 4qc+3
                tag, maxk = PTAGS[qc]
                p = ppool.tile([128, maxk, HPC, 512], BF16, tag=tag, name=tag)
                for ki in range(nki):
                    off = max(0, ki * 128 - qc * 512)
                    sacc = ps_s.tile([128, 2, 512], F32, tag="s")
                    for h in range(HPC):
                        nc.tensor.matmul(
                            sacc[:, h, off:512],
                            kt[h * HD:(h + 1) * HD, ki * 128:(ki + 1) * 128],
                            qt[h * HD:(h + 1) * HD, qc * 512 + off:(qc + 1) * 512],
                            start=True, stop=True)
                    nc.scalar.activation(
                        p[:, ki, :, off:512],
                        sacc[:, :, off:512], EXP, scale=SCALE)
                    if ki >= 4 * qc:  # diagonal: post-exp 0/1 mask per head
                        for h in range(HPC):
                            nc.gpsimd.tensor_mul(
                                p[:, ki, h, off:off + 128],
                                p[:, ki, h, off:off + 128],
                                tri_sb[:])
                return p

            def attn_av(v, ctxt, drows, p, qc):
                nki = 4 * qc + 4
                caccs = [ps_cacc.tile([HD + 1, 512], F32, tag=f"c{h}",
                                      name=f"cacc{h}", bufs=1)
                         for h in range(HPC)]
                for ki in range(nki):
                    off = max(0, ki * 128 - qc * 512)
                    for h in range(HPC):
                        nc.tensor.matmul(
                            caccs[h][:, off:512],
                            v[:, ki, h, :],
                            p[:, ki, h, off:512],
                            start=(ki == 0), stop=(ki == nki - 1))
                for h in range(HPC):
                    dtmp = small.tile([1, 512], F32, tag="dtmp")
                    nc.vector.tensor_copy(dtmp[:], caccs[h][HD:HD + 1, :])
                    nc.gpsimd.dma_start(drows[h][qc:qc + 1, :], dtmp[:])
                    nc.vector.tensor_copy(
                        ctxt[h * HD:(h + 1) * HD, qc * 512:(qc + 1) * 512],
                        caccs[h][0:HD, :])

            def recip_head(drows_h):
                # ACT-table reciprocal (~1e-5 rel err, fine for softmax
                # denominators that land in bf16 anyway): keeps the 3.3us
                # DVE reciprocal out of the serial DVE stream at batch
                # boundaries. bass's activation() hard-blocks Reciprocal,
                # so emit the instruction directly.
                rrows_bf = small.tile([4, 512], BF16, tag="rbf")
                eng = nc.scalar
                ins = [eng.lower_ap(drows_h[:]),
                       mybir.ImmediateValue(dtype=mybir.dt.float32, value=0.0),
                       mybir.ImmediateValue(dtype=mybir.dt.float32, value=1.0),
                       mybir.ImmediateValue(dtype=mybir.dt.float32, value=0.0)]
                eng.add_instruction(mybir.InstActivation(
                    name=nc.get_next_instruction_name(),
                    func=mybir.ActivationFunctionType.Reciprocal,
                    ins=ins, outs=[eng.lower_ap(rrows_bf[:])]))
                return rrows_bf

            def normalize_head(ctxt, rrows_bf, h):
                for qc in range(NQC):
                    bc = ps_mm.tile([128, 512], F32, tag="mm")
                    nc.tensor.matmul(bc[0:HD, :], sel_sb[:, qc, :], rrows_bf[:],
                                     start=True, stop=True)
                    sl = ctxt[h * HD:(h + 1) * HD, qc * 512:(qc + 1) * 512]
                    nc.vector.tensor_mul(sl, sl, bc[0:HD, :])

            def outproj_sb(ctxt, b, sb, alt=False):
                for n in range(2):
                    oacc = ps_mm.tile([128, 512], F32, tag="mm")
                    nc.tensor.matmul(oacc[:],
                                     ctxt[:, sb * 128:(sb + 1) * 128],
                                     wo_sb[:, n * 512:(n + 1) * 512],
                                     start=True, stop=True)
                    osb = outsb.tile([128, 512], BF16, tag="o")
                    if alt and n == 1:
                        nc.scalar.copy(osb[:], oacc[:])
                    else:
                        nc.vector.tensor_copy(osb[:], oacc[:])
                    nc.sync.dma_start(
                        out_d.ap()[b, sb * 128:(sb + 1) * 128,
                                   n * 512:(n + 1) * 512],
                        osb[:])

            xt_sb = load_xt(0)
            prev = None   # (ctxt, rr0, rr1, b) awaiting normalize + outproj
            for b in range(B):
                qt, kt, v = qkv_proj(xt_sb)
                if b + 1 < B:
                    xt_sb = load_xt(b + 1)
                if prev is not None:
                    normalize_head(prev[0], prev[1], 0)
                    normalize_head(prev[0], prev[2], 1)
                    for k in range(4):
                        outproj_sb(prev[0], prev[3], k)
                ctxt = seqpool.tile([128, S], BF16, tag="ctxt")
                drows = [small.tile([4, 512], F32, tag="drows0", name="drows0"),
                         small.tile([4, 512], F32, tag="drows1", name="drows1")]
                pending = None   # (p, qc) with scores emitted, AV not yet
                for qc in range(NQC):
                    p = attn_scores(qt, kt, qc)
                    if pending is not None:
                        attn_av(v, ctxt, drows, pending[0], pending[1])
                        if prev is not None:
                            for k in range(4):
                                outproj_sb(prev[0], prev[3], 4 * qc + k)
                    pending = (p, qc)
                attn_av(v, ctxt, drows, pending[0], pending[1])
                rr0 = recip_head(drows[0])
                rr1 = recip_head(drows[1])
                prev = (ctxt, rr0, rr1, b)
            normalize_head(prev[0], prev[1], 0)
            normalize_head(prev[0], prev[2], 1)
            for sb in range(NSB):
                outproj_sb(prev[0], prev[3], sb, alt=True)
    nc.compile()
    return nc


def _prep_inputs(x, Wq, Wk, Wv, Wo):
    bf16 = ml_dtypes.bfloat16
    xt = np.ascontiguousarray(
        np.asarray(x, dtype=np.float32).reshape(B * S, D).T).astype(bf16)
    k = np.arange(128)[:, None]
    q = np.arange(128)[None, :]
    tri = (q >= k).astype(np.float32).astype(bf16)   # allowed = q >= k
    sel = np.zeros((4, 4, 64), np.float32)
    for r in range(4):
        sel[r, r, :] = 1.0
    sel = sel.reshape(4, 4 * 64).astype(bf16)
    Wq = np.asarray(Wq, dtype=np.float32)
    Wk = np.asarray(Wk, dtype=np.float32)
    Wv = np.asarray(Wv, dtype=np.float32)
    Wo = np.asarray(Wo, dtype=np.float32)
    in_maps = []
    for c in range(N_CORES):
        sl = slice(c * CSL, (c + 1) * CSL)
        in_maps.append({
            "xt": xt,
            "wq": np.ascontiguousarray(Wq[:, sl]).astype(bf16),
            "wk": np.ascontiguousarray(Wk[:, sl]).astype(bf16),
            "wv": np.ascontiguousarray(Wv[:, sl]).astype(bf16),
            "wo": np.ascontiguousarray(Wo[sl, :]).astype(bf16),
            "tri": tri,
            "ident": np.eye(128, dtype=np.float32).astype(bf16),
            "sel": sel,
        })
    return in_maps


def kernel(x, Wq, Wk, Wv, Wo, bo):
    global LAST_RESULTS
    if "nc" not in _CACHE:
        _CACHE["nc"] = _build()
    nc = _CACHE["nc"]
    in_maps = _prep_inputs(x, Wq, Wk, Wv, Wo)
    res = bass_utils.run_bass_kernel_spmd(
        nc, in_maps, core_ids=list(range(N_CORES)))
    LAST_RESULTS = res
    out = np.zeros((B, S, D), dtype=np.float32)
    for r in res.results:
        out += r["out"].astype(np.float32)
    out += np.asarray(bo, dtype=np.float32)
    return out


if __name__ == "__main__":
    rng = np.random.default_rng(0)
    scale = 1.0 / np.sqrt(D)
    ins = {
        "x": rng.standard_normal((B, S, D), dtype=np.float32),
        "Wq": rng.standard_normal((D, D), dtype=np.float32) * scale,
        "Wk": rng.standard_normal((D, D), dtype=np.float32) * scale,
        "Wv": rng.standard_normal((D, D), dtype=np.float32) * scale,
        "Wo": rng.standard_normal((D, D), dtype=np.float32) * scale,
        "bo": np.zeros(D, dtype=np.float32),
    }
    out = kernel(**ins)
    print("kernel output:", out.shape, out.dtype, float(np.abs(out).mean()))


# revision 47
# speedup vs baseline: 1.0239x; 1.0239x over previous
"""Causal multi-head attention on 8 Trainium2 NeuronCores (Bass/Tile).

Problem: B=4, S=2048, D=1024, H=16 heads (HD=64), fp32, causal softmax.

Sharding (tensor parallel over heads): core c owns heads {2c, 2c+1}:
  - Wq/Wk/Wv column slices [D, 128], Wo row slice [128, D]
  - each core computes Q/K/V for its heads over the full batch, runs
    attention for its 8 (batch, head) pairs, and produces a partial
    output projection in bf16, laid out transposed as [B, D, S]; the
    host sums the 8 partials in fp32, transposes to [B, S, D], + bo.

Device dataflow (all matmuls bf16 in / fp32 PSUM accumulate):
  - x is pre-transposed on host to xt[D, B*S] bf16 so Q/K/V projections
    are weight-stationary: Q^T[c, s] = sum_d Wq[d, c] xt[d, s]. V is
    produced transposed and relaid to [s, c] via PE transpose + DVE
    copy.
  - scores are computed transposed, S^T[k, q], with the two heads packed
    onto disjoint PE row-groups (head0 K=64 at partitions 0-63, head1 at
    64-127; confirmed ~80% concurrent on HW); one ACT exp call covers
    both heads' [128, 2, 512] chunk.
  - exp'd scores stream as the AV matmul's moving operand with [V | 1]
    stationary; the ones-column gives softmax denominators in psum row
    64. softmax skips max-subtraction (scaled causal scores are ~N(0,1)).
  - causal masking: sub-diagonal blocks are never computed; diagonal
    128x128 blocks get a post-exp multiplicative 0/1 mask (Pool).
  - denominators: AV evictions drop psum row 64 into a [1, S] row per
    head; one ACT-table reciprocal per (head, batch) on the row (the
    exp<->recip ACT table swap costs ~2.6us, so strictly once per
    batch), then a rank-1 PE matmul (ones-column stationary, K=1)
    broadcasts the recip row to 64 psum partitions and a DVE multiply
    normalizes ctxt in place. The normalize units ride the filler queue
    so the recip latency hides under the next batch's QKV.
  - out^T[d, s] = sum_c Wo[c, d] ctxt[c, s]: weight-stationary out-proj
    in [128, 512] units; psum evictions alternate DVE/ACT.
  - SOFTWARE PIPELINE (engines execute streams in program order): the
    attention of batch b is emitted interleaved with the out-projection
    of batch b-1 (units zipped between score blocks) and the QKV
    projection chunks of batch b+1 (one 512-token chunk per q-chunk
    slot), so the PE never idles while ACT works through exp and the
    clock ramp (0.65/1.2/2.4 GHz pstates, 3us to max) stays hot.
    Batch 0's QKV runs standalone with the xt DMA split into 512-token
    slices so the first matmul starts ~1.5us in; batch 3 normalizes and
    projects per q-chunk right behind its AV to shrink the tail.
"""

import numpy as np
import ml_dtypes

import concourse.bass as bass
import concourse.mybir as mybir
import concourse.tile as tile
from concourse import bacc
from concourse import bass_utils

B, S, D, H, HD = 4, 2048, 1024, 16, 64
N_CORES = 8
HPC = H // N_CORES          # heads per core = 2
CSL = HPC * HD              # per-core channel slice = 128
NSB = S // 128              # 16 s-blocks per sequence
NCH = D // 128              # 8 contraction chunks
NQC = S // 512              # 4 q-chunks of 512
BF16 = mybir.dt.bfloat16
F32 = mybir.dt.float32
EXP = mybir.ActivationFunctionType.Exp
SCALE = 1.0 / float(np.sqrt(HD))

_CACHE: dict = {}
LAST_RESULTS = None  # BassKernelResults of the most recent run (for test.py)


def _build():
    nc = bacc.Bacc("TRN2", target_bir_lowering=False, debug=False,
                   num_devices=N_CORES)
    xt_d = nc.dram_tensor("xt", [D, B * S], BF16, kind="ExternalInput")
    wq_d = nc.dram_tensor("wq", [D, CSL], BF16, kind="ExternalInput")
    wk_d = nc.dram_tensor("wk", [D, CSL], BF16, kind="ExternalInput")
    wv_d = nc.dram_tensor("wv", [D, CSL], BF16, kind="ExternalInput")
    wo_d = nc.dram_tensor("wo", [CSL, D], BF16, kind="ExternalInput")
    tri_d = nc.dram_tensor("tri", [128, 128], BF16, kind="ExternalInput")
    ident_d = nc.dram_tensor("ident", [128, 128], BF16, kind="ExternalInput")
    out_d = nc.dram_tensor("out", [B, D, S], BF16, kind="ExternalOutput")

    with tile.TileContext(nc) as tc:
        with (
            tc.tile_pool(name="const", bufs=1) as cpool,
            tc.tile_pool(name="xt", bufs=2) as xtpool,
            tc.tile_pool(name="seq", bufs=2) as seqpool,
            tc.tile_pool(name="p", bufs=1) as ppool,
            tc.tile_pool(name="small", bufs=4) as small,
            tc.tile_pool(name="outsb", bufs=8) as outsb,
            tc.tile_pool(name="ps_s", bufs=2, space="PSUM") as ps_s,
            tc.tile_pool(name="ps_mm", bufs=2, space="PSUM") as ps_mm,
            tc.tile_pool(name="ps_cacc", bufs=2, space="PSUM") as ps_cacc,
        ):
            wq_sb = cpool.tile([128, NCH, CSL], BF16)
            wk_sb = cpool.tile([128, NCH, CSL], BF16)
            wv_sb = cpool.tile([128, NCH, CSL], BF16)
            wo_sb = cpool.tile([128, NCH, 128], BF16)
            tri_sb = cpool.tile([128, 128], BF16)
            ident_sb = cpool.tile([128, 128], BF16)
            # split the input transfers across BOTH HWDGE queues (sync=SP
            # and scalar=ACT rings run in parallel): dependency granularity
            # is the per-queue completion counter, so what matters is how
            # soon each queue finishes everything ahead of the consumer.
            nc.scalar.dma_start(wq_sb[:], wq_d.ap().rearrange("(c p) m -> p c m", p=128))
            nc.sync.dma_start(wk_sb[:], wk_d.ap().rearrange("(c p) m -> p c m", p=128))
            nc.scalar.dma_start(wv_sb[:], wv_d.ap().rearrange("(c p) m -> p c m", p=128))
            nc.sync.dma_start(wo_sb[:], wo_d.ap().rearrange("p (j m) -> p j m", m=128))
            nc.scalar.dma_start(tri_sb[:], tri_d.ap())
            nc.scalar.dma_start(ident_sb[:], ident_d.ap())

            def load_xt(b, split=False):
                # xt as 4 independent sub-tiles of 2 chunks each: tile-level
                # dependency tracking then lets the first QKV matmul start
                # once the first 1MB sub-tile lands instead of the whole
                # 4MB transfer. split=True also alternates chunks across
                # both HWDGE queues (matters for batch 0).
                xt_sb = [xtpool.tile([128, 2, S], BF16, tag=f"xt{i}",
                                     name=f"xt{i}")
                         for i in range(NCH // 2)]
                for c in range(NCH):
                    eng = nc.scalar if (split and c % 2) else nc.sync
                    eng.dma_start(
                        xt_sb[c // 2][:, c % 2, :],
                        xt_d.ap()[c * 128:(c + 1) * 128, b * S:(b + 1) * S])
                return xt_sb

            def new_seq(b):
                st = {
                    "qt": seqpool.tile([128, S], BF16, tag="qt", name="qt"),
                    "kt": seqpool.tile([128, S], BF16, tag="kt", name="kt"),
                    "vt": seqpool.tile([128, S], BF16, tag="vt", name="vt"),
                    "v": seqpool.tile([128, NSB, HPC, HD + 1], BF16, tag="v",
                                      name="v"),
                }
                nc.vector.memset(st["v"][:, :, :, HD:HD + 1], 1.0)
                return st

            def qkv_chunk(st, xt_sb, sc):
                """Q/K/V projection + V relayout for one 512-token chunk."""
                qkacc = ps_s.tile([128, 2, 512], F32, tag="s", name="qkacc")
                vacc = ps_mm.tile([128, 512], F32, tag="mm", name="vacc")
                for c in range(NCH):
                    xt_c = xt_sb[c // 2][:, c % 2, sc * 512:(sc + 1) * 512]
                    for i, w_sb in enumerate((wq_sb, wk_sb)):
                        nc.tensor.matmul(
                            qkacc[:, i, :],
                            w_sb[:, c, :],
                            xt_c,
                            start=(c == 0), stop=(c == NCH - 1))
                    nc.tensor.matmul(
                        vacc[:],
                        wv_sb[:, c, :],
                        xt_c,
                        start=(c == 0), stop=(c == NCH - 1))
                for i, key in enumerate(("qt", "kt")):
                    nc.vector.tensor_copy(
                        st[key][:, sc * 512:(sc + 1) * 512], qkacc[:, i, :])
                nc.vector.tensor_copy(
                    st["vt"][:, sc * 512:(sc + 1) * 512], vacc[:])
                for i in range(4):
                    sb = sc * 4 + i
                    tp = ps_mm.tile([128, 128], BF16, tag="mm")
                    nc.tensor.transpose(
                        tp[:], st["vt"][:, sb * 128:(sb + 1) * 128], ident_sb[:])
                    nc.vector.tensor_copy(
                        st["v"][:, sb, :, 0:HD],
                        tp[:].rearrange("p (h e) -> p h e", h=HPC))

            # p tiles alternate between two tags (qc0/qc2 and qc1/qc3) so
            # only ~2 q-chunks of exp'd scores are resident at once.
            PTAGS = {0: ("pA", 12), 1: ("pB", 16), 2: ("pA", 12), 3: ("pB", 16)}

            def attn_scores(st, qc, fillers):
                """Score blocks for q-chunk qc; after each block, pop one
                deferred filler (out-proj unit closure) into the PE stream."""
                qt, kt = st["qt"], st["kt"]
                nki = 4 * qc + 4   # causal: k-blocks 0 .. 4qc+3
                tag, maxk = PTAGS[qc]
                p = ppool.tile([128, maxk, HPC, 512], BF16, tag=tag, name=tag)
                for ki in range(nki):
                    off = max(0, ki * 128 - qc * 512)
                    sacc = ps_s.tile([128, 2, 512], F32, tag="s")
                    for h in range(HPC):
                        nc.tensor.matmul(
                            sacc[:, h, off:512],
                            kt[h * HD:(h + 1) * HD, ki * 128:(ki + 1) * 128],
                            qt[h * HD:(h + 1) * HD, qc * 512 + off:(qc + 1) * 512],
                            start=True, stop=True)
                    nc.scalar.activation(
                        p[:, ki, :, off:512],
                        sacc[:, :, off:512], EXP, scale=SCALE)
                    if ki >= 4 * qc:  # diagonal: post-exp 0/1 mask per head
                        for h in range(HPC):
                            nc.gpsimd.tensor_mul(
                                p[:, ki, h, off:off + 128],
                                p[:, ki, h, off:off + 128],
                                tri_sb[:])
                    if fillers:
                        fillers.pop(0)()
                return p

            def attn_av(st, ctxt, drows, p, qc, fillers):
                nki = 4 * qc + 4
                caccs = [ps_cacc.tile([HD + 1, 512], F32, tag=f"c{h}",
                                      name=f"cacc{h}", bufs=1)
                         for h in range(HPC)]
                for ki in range(nki):
                    off = max(0, ki * 128 - qc * 512)
                    for h in range(HPC):
                        nc.tensor.matmul(
                            caccs[h][:, off:512],
                            st["v"][:, ki, h, :],
                            p[:, ki, h, off:512],
                            start=(ki == 0), stop=(ki == nki - 1))
                    if fillers:
                        fillers.pop(0)()
                for h in range(HPC):
                    nc.vector.tensor_copy(
                        drows[h][0:1, qc * 512:(qc + 1) * 512],
                        caccs[h][HD:HD + 1, :])
                    nc.vector.tensor_copy(
                        ctxt[h * HD:(h + 1) * HD, qc * 512:(qc + 1) * 512],
                        caccs[h][0:HD, :])

            def recip_rows(drows_h, rbf_h, c0, c1):
                # ACT-table reciprocal (~1e-5 rel err, fine for softmax
                # denominators that land in bf16 anyway). bass's
                # activation() hard-blocks Reciprocal, so emit directly.
                eng = nc.scalar
                ins = [eng.lower_ap(drows_h[0:1, c0:c1]),
                       mybir.ImmediateValue(dtype=mybir.dt.float32, value=0.0),
                       mybir.ImmediateValue(dtype=mybir.dt.float32, value=1.0),
                       mybir.ImmediateValue(dtype=mybir.dt.float32, value=0.0)]
                eng.add_instruction(mybir.InstActivation(
                    name=nc.get_next_instruction_name(),
                    func=mybir.ActivationFunctionType.Reciprocal,
                    ins=ins, outs=[eng.lower_ap(rbf_h[0:1, c0:c1])]))

            def norm_seg_psum(ctxt, rbf, qc):
                # low-latency rank-1 PE broadcast of the recip row
                # (stationary = ones column from tri row 0, K=1), then
                # per-head DVE multiplies straight off psum (DVE tolerates
                # the psum->sbuf partition shift).
                for h in range(HPC):
                    bc = ps_mm.tile([128, 512], F32, tag="mm", name="bc")
                    nc.tensor.matmul(
                        bc[0:HD, :], tri_sb[0:1, 0:HD],
                        rbf[h][0:1, qc * 512:(qc + 1) * 512],
                        start=True, stop=True)
                    sl = ctxt[h * HD:(h + 1) * HD, qc * 512:(qc + 1) * 512]
                    nc.vector.tensor_mul(sl, sl, bc[0:HD, :])

            def outproj_unit(ctxt, b, j, seg, alt=False):
                oacc = ps_mm.tile([128, 512], F32, tag="mm")
                nc.tensor.matmul(oacc[:],
                                 wo_sb[:, j, :],
                                 ctxt[:, seg * 512:(seg + 1) * 512],
                                 start=True, stop=True)
                osb = outsb.tile([128, 512], BF16, tag="o")
                if alt and j % 2:
                    # last batch only: exp is finished, so ACT can help
                    # drain the epilogue evictions
                    nc.scalar.copy(osb[:], oacc[:])
                else:
                    nc.vector.tensor_copy(osb[:], oacc[:])
                (nc.scalar if (alt and j % 2 == 0) else nc.sync).dma_start(
                    out_d.ap()[b, j * 128:(j + 1) * 128,
                               seg * 512:(seg + 1) * 512],
                    osb[:])

            def build_fillers(ctxt, b, rbf, alt=False):
                """Deferred epilogue of batch b, popped into batch b+1's PE
                stream: 4 normalize units (rank-1 bc + DVE muls) followed by
                32 out-projection units."""
                fs = [lambda qc=qc: norm_seg_psum(ctxt, rbf, qc)
                      for qc in range(NQC)]
                fs += [lambda j=j, seg=seg: outproj_unit(ctxt, b, j, seg, alt)
                       for seg in range(NQC) for j in range(NCH)]
                return fs

            # ---- prologue: batch 0 QKV standalone ----
            xt_sb = load_xt(0, split=True)
            xt_next = load_xt(1)
            seqs = {0: new_seq(0)}
            for sc in range(NQC):
                qkv_chunk(seqs[0], xt_sb, sc)
            xt_sb = xt_next

            prev = None   # (ctxt of b-1, b-1) with outproj not yet emitted
            for b in range(B):
                st = seqs.pop(b)
                ctxt = seqpool.tile([128, S], BF16, tag="ctxt")
                drows = [small.tile([1, S], F32, tag="drows0", name="drows0",
                                    bufs=1),
                         small.tile([1, S], F32, tag="drows1", name="drows1",
                                    bufs=1)]
                rbf = [small.tile([1, S], BF16, tag="rbf0", name="rbf0",
                                  bufs=1),
                       small.tile([1, S], BF16, tag="rbf1", name="rbf1",
                                  bufs=1)]
                fillers = build_fillers(*prev) if prev else []

                # hold back a few of b-1's out-proj units: the last batch
                # needs PE cover for its reciprocal chain after the final AV
                reserve = []
                if b == B - 1 and len(fillers) >= 8:
                    reserve = fillers[-8:]
                    del fillers[-8:]

                pending = None   # (p, qc) with scores emitted, AV not yet
                for qc in range(NQC):
                    if b + 1 < B:
                        if qc == 0:
                            seqs[b + 1] = new_seq(b + 1)
                        qkv_chunk(seqs[b + 1], xt_sb, qc)
                    # slot 0 pops nothing: the b-1 epilogue's reciprocal
                    # (ACT table swap) hasn't resolved yet and the norm
                    # units at the head of the queue would stall the PE
                    p = attn_scores(st, qc, fillers if qc > 0 else [])
                    if pending is not None:
                        attn_av(st, ctxt, drows, pending[0], pending[1], fillers)
                    pending = (p, qc)
                    if qc == 2 and b + 2 < B:
                        xt_next = load_xt(b + 2)
                attn_av(st, ctxt, drows, pending[0], pending[1], fillers)
                for f in fillers:   # leftover units of b-1
                    f()
                for h in range(HPC):
                    recip_rows(drows[h], rbf[h], 0, S)
                for f in reserve:   # PE cover while the recip chain runs
                    f()
                if b == B - 1:
                    # inline epilogue for the last batch
                    for f in build_fillers(ctxt, b, rbf, alt=True):
                        f()
                else:
                    prev = (ctxt, b, rbf)
                    xt_sb = xt_next
    nc.compile()
    return nc


def _prep_inputs(x, Wq, Wk, Wv, Wo):
    bf16 = ml_dtypes.bfloat16
    xt = np.ascontiguousarray(
        np.asarray(x, dtype=np.float32).reshape(B * S, D).T).astype(bf16)
    k = np.arange(128)[:, None]
    q = np.arange(128)[None, :]
    tri = (q >= k).astype(np.float32).astype(bf16)   # allowed = q >= k
    Wq = np.asarray(Wq, dtype=np.float32)
    Wk = np.asarray(Wk, dtype=np.float32)
    Wv = np.asarray(Wv, dtype=np.float32)
    Wo = np.asarray(Wo, dtype=np.float32)
    in_maps = []
    for c in range(N_CORES):
        sl = slice(c * CSL, (c + 1) * CSL)
        in_maps.append({
            "xt": xt,
            "wq": np.ascontiguousarray(Wq[:, sl]).astype(bf16),
            "wk": np.ascontiguousarray(Wk[:, sl]).astype(bf16),
            "wv": np.ascontiguousarray(Wv[:, sl]).astype(bf16),
            "wo": np.ascontiguousarray(Wo[sl, :]).astype(bf16),
            "tri": tri,
            "ident": np.eye(128, dtype=np.float32).astype(bf16),
        })
    return in_maps


def kernel(x, Wq, Wk, Wv, Wo, bo):
    global LAST_RESULTS
    if "nc" not in _CACHE:
        _CACHE["nc"] = _build()
    nc = _CACHE["nc"]
    in_maps = _prep_inputs(x, Wq, Wk, Wv, Wo)
    res = bass_utils.run_bass_kernel_spmd(
        nc, in_maps, core_ids=list(range(N_CORES)))
    LAST_RESULTS = res
    acc = np.zeros((B, D, S), dtype=np.float32)
    for r in res.results:
        acc += r["out"].astype(np.float32)
    out = np.ascontiguousarray(acc.transpose(0, 2, 1))
    out += np.asarray(bo, dtype=np.float32)
    return out


if __name__ == "__main__":
    rng = np.random.default_rng(0)
    scale = 1.0 / np.sqrt(D)
    ins = {
        "x": rng.standard_normal((B, S, D), dtype=np.float32),
        "Wq": rng.standard_normal((D, D), dtype=np.float32) * scale,
        "Wk": rng.standard_normal((D, D), dtype=np.float32) * scale,
        "Wv": rng.standard_normal((D, D), dtype=np.float32) * scale,
        "Wo": rng.standard_normal((D, D), dtype=np.float32) * scale,
        "bo": np.zeros(D, dtype=np.float32),
    }
    out = kernel(**ins)
    print("kernel output:", out.shape, out.dtype, float(np.abs(out).mean()))


# revision 54
# speedup vs baseline: 1.0326x; 1.0085x over previous
"""Causal multi-head attention on 8 Trainium2 NeuronCores (Bass/Tile).

Problem: B=4, S=2048, D=1024, H=16 heads (HD=64), fp32, causal softmax.

Sharding (tensor parallel over heads): core c owns heads {2c, 2c+1}:
  - Wq/Wk/Wv column slices [D, 128], Wo row slice [128, D]
  - each core computes Q/K/V for its heads over the full batch, runs
    attention for its 8 (batch, head) pairs, and produces a partial
    output projection in bf16, laid out transposed as [B, D, S]; the
    host sums the 8 partials in fp32, transposes to [B, S, D], + bo.

Device dataflow (all matmuls bf16 in / fp32 PSUM accumulate):
  - x is pre-transposed on host to xt[D, B*S] bf16 so Q/K/V projections
    are weight-stationary: Q^T[c, s] = sum_d Wq[d, c] xt[d, s]. V is
    produced transposed and relaid to [s, c] via PE transpose + DVE
    copy.
  - scores are computed transposed, S^T[k, q], with the two heads packed
    onto disjoint PE row-groups (head0 K=64 at partitions 0-63, head1 at
    64-127; confirmed ~80% concurrent on HW); one ACT exp call covers
    both heads' [128, 2, 512] chunk.
  - exp'd scores stream as the AV matmul's moving operand with [V | 1]
    stationary; the ones-column gives softmax denominators in psum row
    64. softmax skips max-subtraction (scaled causal scores are ~N(0,1)).
  - causal masking: sub-diagonal blocks are never computed; diagonal
    128x128 blocks get a post-exp multiplicative 0/1 mask (Pool).
  - denominators: AV evictions drop psum row 64 into a [1, S] row per
    head; one ACT-table reciprocal per (head, batch) on the row (the
    exp<->recip ACT table swap costs ~2.6us, so strictly once per
    batch), then a rank-1 PE matmul (ones-column stationary, K=1)
    broadcasts the recip row to 64 psum partitions and a DVE multiply
    normalizes ctxt in place. The normalize units ride the filler queue
    so the recip latency hides under the next batch's QKV.
  - out^T[d, s] = sum_c Wo[c, d] ctxt[c, s]: weight-stationary out-proj
    in [128, 512] units; psum evictions alternate DVE/ACT.
  - SOFTWARE PIPELINE (engines execute streams in program order): the
    attention of batch b is emitted interleaved with the out-projection
    of batch b-1 (units zipped between score blocks) and the QKV
    projection chunks of batch b+1 (one 512-token chunk per q-chunk
    slot), so the PE never idles while ACT works through exp and the
    clock ramp (0.65/1.2/2.4 GHz pstates, 3us to max) stays hot.
    Batch 0's QKV runs standalone with the xt DMA split into 512-token
    slices so the first matmul starts ~1.5us in; batch 3 normalizes and
    projects per q-chunk right behind its AV to shrink the tail.
"""

import numpy as np
import ml_dtypes

import concourse.bass as bass
import concourse.mybir as mybir
import concourse.tile as tile
from concourse import bacc
from concourse import bass_utils

B, S, D, H, HD = 4, 2048, 1024, 16, 64
N_CORES = 8
HPC = H // N_CORES          # heads per core = 2
CSL = HPC * HD              # per-core channel slice = 128
NSB = S // 128              # 16 s-blocks per sequence
NCH = D // 128              # 8 contraction chunks
NQC = S // 512              # 4 q-chunks of 512
BF16 = mybir.dt.bfloat16
F32 = mybir.dt.float32
EXP = mybir.ActivationFunctionType.Exp
SCALE = 1.0 / float(np.sqrt(HD))

_CACHE: dict = {}
LAST_RESULTS = None  # BassKernelResults of the most recent run (for test.py)


def _build():
    nc = bacc.Bacc("TRN2", target_bir_lowering=False, debug=False,
                   num_devices=N_CORES)
    xt_d = nc.dram_tensor("xt", [D, B * S], BF16, kind="ExternalInput")
    # wq/wk/wv arrive host-pre-transposed to the stationary layout
    # [128, NCH*CSL] so their DMAs are fully contiguous (the natural
    # [D, CSL] layout needs 256B-packet strided DMAs that take ~10us
    # and gate the very first matmul)
    wq_d = nc.dram_tensor("wq", [128, NCH * CSL], BF16, kind="ExternalInput")
    wk_d = nc.dram_tensor("wk", [128, NCH * CSL], BF16, kind="ExternalInput")
    wv_d = nc.dram_tensor("wv", [128, NCH * CSL], BF16, kind="ExternalInput")
    wo_d = nc.dram_tensor("wo", [CSL, D], BF16, kind="ExternalInput")
    tri_d = nc.dram_tensor("tri", [128, 128], BF16, kind="ExternalInput")
    ident_d = nc.dram_tensor("ident", [128, 128], BF16, kind="ExternalInput")
    out_d = nc.dram_tensor("out", [B, D, S], BF16, kind="ExternalOutput")

    with tile.TileContext(nc) as tc:
        with (
            tc.tile_pool(name="const", bufs=1) as cpool,
            tc.tile_pool(name="xt", bufs=2) as xtpool,
            tc.tile_pool(name="seq", bufs=2) as seqpool,
            tc.tile_pool(name="p", bufs=1) as ppool,
            tc.tile_pool(name="small", bufs=4) as small,
            tc.tile_pool(name="outsb", bufs=10) as outsb,
            tc.tile_pool(name="ps_s", bufs=2, space="PSUM") as ps_s,
            tc.tile_pool(name="ps_mm", bufs=2, space="PSUM") as ps_mm,
            tc.tile_pool(name="ps_cacc", bufs=2, space="PSUM") as ps_cacc,
        ):
            wq_sb = cpool.tile([128, NCH, CSL], BF16)
            wk_sb = cpool.tile([128, NCH, CSL], BF16)
            wv_sb = cpool.tile([128, NCH, CSL], BF16)
            wo_sb = cpool.tile([128, NCH, 128], BF16)
            tri_sb = cpool.tile([128, 128], BF16)
            ident_sb = cpool.tile([128, 128], BF16)
            # split the input transfers across BOTH HWDGE queues (sync=SP
            # and scalar=ACT rings run in parallel): dependency granularity
            # is the per-queue completion counter, so what matters is how
            # soon each queue finishes everything ahead of the consumer.
            nc.scalar.dma_start(wq_sb[:], wq_d.ap().rearrange("p (c m) -> p c m", m=CSL))
            nc.sync.dma_start(wk_sb[:], wk_d.ap().rearrange("p (c m) -> p c m", m=CSL))
            nc.scalar.dma_start(wv_sb[:], wv_d.ap().rearrange("p (c m) -> p c m", m=CSL))
            nc.sync.dma_start(wo_sb[:], wo_d.ap().rearrange("p (j m) -> p j m", m=128))
            nc.scalar.dma_start(tri_sb[:], tri_d.ap())
            nc.scalar.dma_start(ident_sb[:], ident_d.ap())

            def load_xt(b, split=False):
                # xt as 4 independent sub-tiles of 2 chunks each: tile-level
                # dependency tracking then lets the first QKV matmul start
                # once the first 1MB sub-tile lands instead of the whole
                # 4MB transfer. split=True also alternates chunks across
                # both HWDGE queues (matters for batch 0).
                xt_sb = [xtpool.tile([128, 2, S], BF16, tag=f"xt{i}",
                                     name=f"xt{i}")
                         for i in range(NCH // 2)]
                for c in range(NCH):
                    eng = nc.scalar if (split and c % 2) else nc.sync
                    eng.dma_start(
                        xt_sb[c // 2][:, c % 2, :],
                        xt_d.ap()[c * 128:(c + 1) * 128, b * S:(b + 1) * S])
                return xt_sb

            def new_seq(b):
                st = {
                    "qt": seqpool.tile([128, S], BF16, tag="qt", name="qt"),
                    "kt": seqpool.tile([128, S], BF16, tag="kt", name="kt"),
                    "vt": seqpool.tile([128, S], BF16, tag="vt", name="vt"),
                    "v": seqpool.tile([128, NSB, HPC, HD + 1], BF16, tag="v",
                                      name="v"),
                }
                nc.vector.memset(st["v"][:, :, :, HD:HD + 1], 1.0)
                return st

            def qkv_chunk(st, xt_sb, sc):
                """Q/K/V projection + V relayout for one 512-token chunk."""
                qkacc = ps_s.tile([128, 2, 512], F32, tag="s", name="qkacc")
                vacc = ps_mm.tile([128, 512], F32, tag="mm", name="vacc")
                for c in range(NCH):
                    xt_c = xt_sb[c // 2][:, c % 2, sc * 512:(sc + 1) * 512]
                    for i, w_sb in enumerate((wq_sb, wk_sb)):
                        nc.tensor.matmul(
                            qkacc[:, i, :],
                            w_sb[:, c, :],
                            xt_c,
                            start=(c == 0), stop=(c == NCH - 1))
                    nc.tensor.matmul(
                        vacc[:],
                        wv_sb[:, c, :],
                        xt_c,
                        start=(c == 0), stop=(c == NCH - 1))
                for i, key in enumerate(("qt", "kt")):
                    nc.vector.tensor_copy(
                        st[key][:, sc * 512:(sc + 1) * 512], qkacc[:, i, :])
                nc.vector.tensor_copy(
                    st["vt"][:, sc * 512:(sc + 1) * 512], vacc[:])
                for i in range(4):
                    sb = sc * 4 + i
                    tp = ps_mm.tile([128, 128], BF16, tag="mm")
                    nc.tensor.transpose(
                        tp[:], st["vt"][:, sb * 128:(sb + 1) * 128], ident_sb[:])
                    nc.vector.tensor_copy(
                        st["v"][:, sb, :, 0:HD],
                        tp[:].rearrange("p (h e) -> p h e", h=HPC))

            # p tiles alternate between two tags (qc0/qc2 and qc1/qc3) so
            # only ~2 q-chunks of exp'd scores are resident at once.
            PTAGS = {0: ("pA", 12), 1: ("pB", 16), 2: ("pA", 12), 3: ("pB", 16)}

            def attn_scores(st, qc, fillers):
                """Score blocks for q-chunk qc; after each block, pop one
                deferred filler (out-proj unit closure) into the PE stream."""
                qt, kt = st["qt"], st["kt"]
                nki = 4 * qc + 4   # causal: k-blocks 0 .. 4qc+3
                tag, maxk = PTAGS[qc]
                p = ppool.tile([128, maxk, HPC, 512], BF16, tag=tag, name=tag)
                for ki in range(nki):
                    off = max(0, ki * 128 - qc * 512)
                    sacc = ps_s.tile([128, 2, 512], F32, tag="s")
                    for h in range(HPC):
                        nc.tensor.matmul(
                            sacc[:, h, off:512],
                            kt[h * HD:(h + 1) * HD, ki * 128:(ki + 1) * 128],
                            qt[h * HD:(h + 1) * HD, qc * 512 + off:(qc + 1) * 512],
                            start=True, stop=True)
                    nc.scalar.activation(
                        p[:, ki, :, off:512],
                        sacc[:, :, off:512], EXP, scale=SCALE)
                    if ki >= 4 * qc:  # diagonal: post-exp 0/1 mask per head
                        for h in range(HPC):
                            nc.gpsimd.tensor_mul(
                                p[:, ki, h, off:off + 128],
                                p[:, ki, h, off:off + 128],
                                tri_sb[:])
                    if fillers:
                        fillers.pop(0)()
                return p

            def attn_av(st, ctxt, drows, p, qc, fillers):
                nki = 4 * qc + 4
                caccs = [ps_cacc.tile([HD + 1, 512], F32, tag=f"c{h}",
                                      name=f"cacc{h}", bufs=1)
                         for h in range(HPC)]
                for ki in range(nki):
                    off = max(0, ki * 128 - qc * 512)
                    for h in range(HPC):
                        nc.tensor.matmul(
                            caccs[h][:, off:512],
                            st["v"][:, ki, h, :],
                            p[:, ki, h, off:512],
                            start=(ki == 0), stop=(ki == nki - 1))
                    if fillers:
                        fillers.pop(0)()
                for h in range(HPC):
                    nc.vector.tensor_copy(
                        drows[h][0:1, qc * 512:(qc + 1) * 512],
                        caccs[h][HD:HD + 1, :])
                    nc.vector.tensor_copy(
                        ctxt[h * HD:(h + 1) * HD, qc * 512:(qc + 1) * 512],
                        caccs[h][0:HD, :])

            def recip_rows(drows_h, rbf_h, c0, c1):
                # ACT-table reciprocal (~1e-5 rel err, fine for softmax
                # denominators that land in bf16 anyway). bass's
                # activation() hard-blocks Reciprocal, so emit directly.
                eng = nc.scalar
                ins = [eng.lower_ap(drows_h[0:1, c0:c1]),
                       mybir.ImmediateValue(dtype=mybir.dt.float32, value=0.0),
                       mybir.ImmediateValue(dtype=mybir.dt.float32, value=1.0),
                       mybir.ImmediateValue(dtype=mybir.dt.float32, value=0.0)]
                eng.add_instruction(mybir.InstActivation(
                    name=nc.get_next_instruction_name(),
                    func=mybir.ActivationFunctionType.Reciprocal,
                    ins=ins, outs=[eng.lower_ap(rbf_h[0:1, c0:c1])]))

            def norm_seg_psum(ctxt, rbf, qc):
                # low-latency rank-1 PE broadcast of the recip row
                # (stationary = ones column from tri row 0, K=1), then
                # per-head DVE multiplies straight off psum (DVE tolerates
                # the psum->sbuf partition shift).
                for h in range(HPC):
                    bc = ps_mm.tile([128, 512], F32, tag="mm", name="bc")
                    nc.tensor.matmul(
                        bc[0:HD, :], tri_sb[0:1, 0:HD],
                        rbf[h][0:1, qc * 512:(qc + 1) * 512],
                        start=True, stop=True)
                    sl = ctxt[h * HD:(h + 1) * HD, qc * 512:(qc + 1) * 512]
                    nc.vector.tensor_mul(sl, sl, bc[0:HD, :])

            def outproj_unit(ctxt, b, j, seg, alt=False):
                oacc = ps_mm.tile([128, 512], F32, tag="mm")
                nc.tensor.matmul(oacc[:],
                                 wo_sb[:, j, :],
                                 ctxt[:, seg * 512:(seg + 1) * 512],
                                 start=True, stop=True)
                osb = outsb.tile([128, 512], BF16, tag="o")
                if alt and j % 2:
                    # last batch only: exp is finished, so ACT can help
                    # drain the epilogue evictions
                    nc.scalar.copy(osb[:], oacc[:])
                else:
                    nc.vector.tensor_copy(osb[:], oacc[:])
                (nc.scalar if (alt and j % 2 == 0) else nc.sync).dma_start(
                    out_d.ap()[b, j * 128:(j + 1) * 128,
                               seg * 512:(seg + 1) * 512],
                    osb[:])

            def build_fillers(ctxt, b, rbf, alt=False):
                """Deferred epilogue of batch b, popped into batch b+1's PE
                stream: 4 normalize units (rank-1 bc + DVE muls) followed by
                32 out-projection units."""
                fs = [lambda qc=qc: norm_seg_psum(ctxt, rbf, qc)
                      for qc in range(NQC)]
                fs += [lambda j=j, seg=seg: outproj_unit(ctxt, b, j, seg, alt)
                       for seg in range(NQC) for j in range(NCH)]
                return fs

            # ---- prologue: batch 0 QKV standalone ----
            xt_sb = load_xt(0, split=True)
            xt_next = load_xt(1)
            seqs = {0: new_seq(0)}
            for sc in range(NQC):
                qkv_chunk(seqs[0], xt_sb, sc)
            xt_sb = xt_next

            prev = None   # (ctxt of b-1, b-1) with outproj not yet emitted
            for b in range(B):
                st = seqs.pop(b)
                ctxt = seqpool.tile([128, S], BF16, tag="ctxt")
                drows = [small.tile([1, S], F32, tag="drows0", name="drows0",
                                    bufs=1),
                         small.tile([1, S], F32, tag="drows1", name="drows1",
                                    bufs=1)]
                rbf = [small.tile([1, S], BF16, tag="rbf0", name="rbf0",
                                  bufs=1),
                       small.tile([1, S], BF16, tag="rbf1", name="rbf1",
                                  bufs=1)]
                fillers = build_fillers(*prev) if prev else []

                # hold back a few of b-1's out-proj units: the last batch
                # needs PE cover for its reciprocal chain after the final AV
                reserve = []
                if b == B - 1 and len(fillers) >= 8:
                    # re-wrap the reserved units with alt=True: they run in
                    # the tail where ACT/scalar-queue are free to help
                    rs = NQC - 1
                    reserve = [lambda j=j: outproj_unit(prev[0], prev[1], j,
                                                        rs, alt=True)
                               for j in range(NCH)]
                    del fillers[-8:]

                pending = None   # (p, qc) with scores emitted, AV not yet
                for qc in range(NQC):
                    if b + 1 < B:
                        if qc == 0:
                            seqs[b + 1] = new_seq(b + 1)
                        qkv_chunk(seqs[b + 1], xt_sb, qc)
                    # early slots pop nothing: the b-1 epilogue's reciprocal
                    # (ACT table swap) hasn't resolved yet and the norm
                    # units at the head of the queue would stall the PE.
                    # batch 3 has no QKV cover, so it waits one slot longer.
                    popok = qc >= (2 if b == B - 1 else 1)
                    p = attn_scores(st, qc, fillers if popok else [])
                    if pending is not None:
                        attn_av(st, ctxt, drows, pending[0], pending[1],
                                fillers if popok else [])
                    pending = (p, qc)
                    if qc == 2 and b + 2 < B:
                        xt_next = load_xt(b + 2)
                attn_av(st, ctxt, drows, pending[0], pending[1], fillers)
                for f in fillers:   # leftover units of b-1
                    f()
                for h in range(HPC):
                    recip_rows(drows[h], rbf[h], 0, S)
                for f in reserve:   # PE cover while the recip chain runs
                    f()
                if b == B - 1:
                    # inline epilogue for the last batch
                    for f in build_fillers(ctxt, b, rbf, alt=True):
                        f()
                else:
                    prev = (ctxt, b, rbf)
                    xt_sb = xt_next
    nc.compile()
    return nc


def _prep_inputs(x, Wq, Wk, Wv, Wo):
    bf16 = ml_dtypes.bfloat16
    xt = np.ascontiguousarray(
        np.asarray(x, dtype=np.float32).reshape(B * S, D).T).astype(bf16)
    k = np.arange(128)[:, None]
    q = np.arange(128)[None, :]
    tri = (q >= k).astype(np.float32).astype(bf16)   # allowed = q >= k
    Wq = np.asarray(Wq, dtype=np.float32)
    Wk = np.asarray(Wk, dtype=np.float32)
    Wv = np.asarray(Wv, dtype=np.float32)
    Wo = np.asarray(Wo, dtype=np.float32)
    in_maps = []
    for c in range(N_CORES):
        sl = slice(c * CSL, (c + 1) * CSL)
        def wlayout(w):
            # [D, CSL] -> stationary layout [128, NCH*CSL]:
            # out[p, c*CSL+m] = w[c*128+p, m]
            return np.ascontiguousarray(
                w.reshape(NCH, 128, CSL).transpose(1, 0, 2)
                .reshape(128, NCH * CSL)).astype(bf16)

        in_maps.append({
            "xt": xt,
            "wq": wlayout(Wq[:, sl]),
            "wk": wlayout(Wk[:, sl]),
            "wv": wlayout(Wv[:, sl]),
            "wo": np.ascontiguousarray(Wo[sl, :]).astype(bf16),
            "tri": tri,
            "ident": np.eye(128, dtype=np.float32).astype(bf16),
        })
    return in_maps


def kernel(x, Wq, Wk, Wv, Wo, bo):
    global LAST_RESULTS
    if "nc" not in _CACHE:
        _CACHE["nc"] = _build()
    nc = _CACHE["nc"]
    in_maps = _prep_inputs(x, Wq, Wk, Wv, Wo)
    res = bass_utils.run_bass_kernel_spmd(
        nc, in_maps, core_ids=list(range(N_CORES)))
    LAST_RESULTS = res
    acc = np.zeros((B, D, S), dtype=np.float32)
    for r in res.results:
        acc += r["out"].astype(np.float32)
    out = np.ascontiguousarray(acc.transpose(0, 2, 1))
    out += np.asarray(bo, dtype=np.float32)
    return out


if __name__ == "__main__":
    rng = np.random.default_rng(0)
    scale = 1.0 / np.sqrt(D)
    ins = {
        "x": rng.standard_normal((B, S, D), dtype=np.float32),
        "Wq": rng.standard_normal((D, D), dtype=np.float32) * scale,
        "Wk": rng.standard_normal((D, D), dtype=np.float32) * scale,
        "Wv": rng.standard_normal((D, D), dtype=np.float32) * scale,
        "Wo": rng.standard_normal((D, D), dtype=np.float32) * scale,
        "bo": np.zeros(D, dtype=np.float32),
    }
    out = kernel(**ins)
    print("kernel output:", out.shape, out.dtype, float(np.abs(out).mean()))
